# revision 34
# baseline (speedup 1.0000x reference)
"""MoE layer (16 experts, top-2, SwiGLU FFN + shared expert) on 8 Trainium2 cores.

Strategy (expert-parallel, host-side dispatch):
  - Host computes the gate (logits -> softmax -> top-2 -> combine weights) in
    float64 numpy; the min 2nd-vs-3rd logit gap at these scales is ~1e-4, so
    the top-2 set matches the f32 reference selection robustly.
  - Core m owns experts 2m and 2m+1.  Tokens routed to an expert are gathered,
    transposed to feature-major [D, T] and padded to a fixed capacity C.
  - Each core also runs the shared expert on a 1/8 slice of all tokens.
  - The device kernel does three dense SwiGLU blocks (expert A, expert B,
    shared slice) in feature-major layout: all matmuls keep weights stationary
    ([K=128 d, M=128 f] tiles) and stream token columns, so no on-chip
    transposes are needed.  Matmuls run in bf16 (full PE rate, half the DMA
    traffic of fp32; PSUM accumulation stays fp32).
  - Host applies the combine weights and scatter-adds expert contributions in
    expert-id order (matching the reference's accumulation order), then adds
    the shared-expert output.
"""

import numpy as np

E, TOPK, D, F = 16, 2, 2048, 1408
B, S = 4, 2048
N = B * S
NCORES = 8
C = 1024               # per-expert token capacity (multiple of 512 keeps all
                       # matmul chunks at N=512, one PSUM bank; the rare
                       # overflow tokens are finished on the host)
NS = N // NCORES       # shared-expert tokens per core
TBLOCKS = [(0, C), (C, C), (2 * C, NS)]   # (col offset, ncols) per swiglu block
TCOLS = 2 * C + NS
KD = D // 128          # 16 k-tiles over D
MF = F // 128          # 11 m-tiles over F

_prog_cache = {}


def _emit(ctx, tc, xT, gup, wdp, yT, xq8, gup8):
    import concourse.bass as bass  # noqa: F401
    from concourse import mybir

    nc = tc.nc
    f32 = mybir.dt.float32
    bf16 = mybir.dt.bfloat16
    f8 = mybir.dt.float8e4
    silu = mybir.ActivationFunctionType.Silu
    copyf = mybir.ActivationFunctionType.Copy
    DR = mybir.MatmulPerfMode.DoubleRow

    xpool = ctx.enter_context(tc.tile_pool(name="x", bufs=1))
    hpool = ctx.enter_context(tc.tile_pool(name="h", bufs=2))
    wpool = ctx.enter_context(tc.tile_pool(name="w", bufs=3))
    opool = ctx.enter_context(tc.tile_pool(name="o", bufs=4))
    tpool = ctx.enter_context(tc.tile_pool(name="t", bufs=2))
    pspool = ctx.enter_context(tc.tile_pool(name="ps", bufs=1, space="PSUM"))

    # PSUM bank plan (8 banks, one [128,512] f32 tile each).  Phase-1
    # accumulators alternate tag sets by m-parity so the first matmuls of
    # pass m+1 never wait on pass m's activation/mul reads (W-A-R).
    P1TAGS = [["psA", "psB", "psC", "psD"], ["psE", "psF", "psG", "psH"]]
    # Phase-2 accumulators alternate by m2-parity for the same reason and
    # reuse phase-1 banks (Tile sequences the W-A-R hand-off).
    P2TAGS = [["psA", "psC"], ["psE", "psG"]]

    # PE warm-up: dummy matmuls bridge until the first x/weight DMAs land
    # (~15us: ring ramp-up) and release the HAM clock-gate (1.2->2.4GHz)
    # before real work starts.
    zsrc = wpool.tile([128, 512], bf16, tag="warm", bufs=1, name="zsrc")
    nc.vector.memset(zsrc[:], 0.0)
    pwarm = pspool.tile([128, 512], f32, tag="psH", name="pwarm")
    for i in range(19):
        nc.tensor.matmul(pwarm[:, :], zsrc[:, 0:128], zsrc[:, :],
                         start=(i == 0), stop=(i == 18))

    xtiles = {}
    wpre = {}

    def _issue_x_piece(b, k0, k1):
        off, T = TBLOCKS[b]
        if b not in xtiles:
            xtiles[b] = xpool.tile([128, KD, T], bf16, tag="x", name=f"x{b}")
        nc.sync.dma_start(xtiles[b][:, k0:k1, :], xT[:, k0:k1, off:off + T])

    def _issue_w(b, m):
        w_sb = wpool.tile([128, KD * 256], bf16, tag="w", name=f"w{b}_{m}")
        nc.sync.dma_start(w_sb[:, :], gup[b, m, :, :])
        wpre[(b, m)] = w_sb

    for b, (off, T) in enumerate(TBLOCKS):
        nch = T // 512                      # all chunks are exactly 512 wide
        chunks = [c0 * 512 for c0 in range(nch)]

        # Each dma_start costs ~650ns of serialized dispatch on the sync
        # queue (SWDGE first-byte), so transfers are batched into few large
        # access patterns; xT is laid out partition-major [128, KD, TCOLS]
        # so a k-group x load is one 2D-per-partition pattern.  Blocks b>0
        # have their x and m=0/1 weights prefetched during block b-1's
        # phase 2 (sync-queue slots permitting) so transitions are seamless.
        w0 = {}
        if b == 0:
            # the head is paced by the DMA-ring ramp: pieces are issued in
            # exactly the order the m=0 sub-phases consume them
            x_sb = xpool.tile([128, KD, T], bf16, tag="x", name="x0")
            xtiles[0] = x_sb
            kq = KD // 4
            w0[0] = wpool.tile([128, (KD // 2) * 256], bf16, tag="w", name="w0_0_0")
            nc.sync.dma_start(w0[0][:, :], gup[b, 0, :, 0:(KD // 2) * 256])
            w0[1] = wpool.tile([128, (KD // 2) * 256], bf16, tag="w", name="w0_0_1")
            # arrival order matches the m=0 sub-phase consumption order below
            for (ci, q) in ((0, 0), (1, 0), (0, 1), (1, 1)):
                nc.sync.dma_start(
                    x_sb[:, q * 2 * kq:(q * 2 + 1) * kq, ci * 512:ci * 512 + 512],
                    xT[:, q * 2 * kq:(q * 2 + 1) * kq, off + ci * 512:off + ci * 512 + 512])
                nc.sync.dma_start(
                    x_sb[:, (q * 2 + 1) * kq:(q * 2 + 2) * kq, ci * 512:ci * 512 + 512],
                    xT[:, (q * 2 + 1) * kq:(q * 2 + 2) * kq, off + ci * 512:off + ci * 512 + 512])
                if ci == 0 and q == 0:
                    nc.sync.dma_start(w0[1][:, :], gup[b, 0, :, (KD // 2) * 256:KD * 256])
            # fp8 copy of this block's ci=1 columns (used by m>=1 DoubleRow):
            # cast on-device from the bf16 x already in flight, so the 1MB
            # doesn't compete for ring bandwidth in the ramp-limited head
            x8_sb = xpool.tile([128, 8, 2, 512], f8, tag="x8a", name="x8a")
            for kk8 in range(8):
                for j8 in range(2):
                    nc.vector.tensor_copy(x8_sb[:, kk8, j8, :],
                                          x_sb[:, 2 * kk8 + j8, 512:1024])
        else:
            x_sb = xtiles[b]
            if b == 1:
                x8_sb = xpool.tile([128, 8, 2, 512], f8, tag="x8b", name="x8b")
                nc.sync.dma_start(x8_sb[:], xq8[1])

        h_sb = hpool.tile([128, MF, T], bf16, tag="h", name=f"h{b}")

        # phase 1: gate/up matmuls + silu*up -> h
        for m in range(MF):
            tg = P1TAGS[m % 2]
            pg = [pspool.tile([128, 512], f32, tag=tg[ci], name=f"pg{b}_{m}_{ci}")
                  for ci in range(nch)]
            pu = [pspool.tile([128, 512], f32, tag=tg[2 + ci], name=f"pu{b}_{m}_{ci}")
                  for ci in range(nch)]

            wm_full = None
            if not (b == 0 and m == 0):
                if (b, m) in wpre:
                    wm_full = wpre.pop((b, m))
                else:
                    _issue_w(b, m)
                    wm_full = wpre.pop((b, m))

            def _w_for(half):
                if b == 0 and m == 0:
                    return w0[half]
                return wm_full[:, half * (KD // 2) * 256:(half + 1) * (KD // 2) * 256]

            if b == 0 and m == 0:
                # k-half x ci sub-phases in DMA arrival order, so the PE
                # streams right behind the (ramping) rings with no stalls
                wtiles = [_w_for(0), _w_for(1)]
                for ci, kr in ((0, range(0, KD // 2)), (1, range(0, KD // 2)),
                               (0, range(KD // 2, KD)), (1, range(KD // 2, KD))):
                    c0 = chunks[ci]
                    for k in kr:
                        w_sb = wtiles[k // (KD // 2)]
                        j = k % (KD // 2)
                        lg = w_sb[:, (2 * j) * 128:(2 * j + 1) * 128]
                        lu = w_sb[:, (2 * j + 1) * 128:(2 * j + 2) * 128]
                        nc.tensor.matmul(pg[ci][:, :], lg,
                                         x_sb[:, k, c0:c0 + 512],
                                         start=(k == 0), stop=(k == KD - 1))
                        nc.tensor.matmul(pu[ci][:, :], lu,
                                         x_sb[:, k, c0:c0 + 512],
                                         start=(k == 0), stop=(k == KD - 1))
                    if kr.stop == KD:
                        sil = tpool.tile([128, 512], f32, tag="t", name=f"s{b}_{m}_{ci}")
                        nc.scalar.activation(sil[:], pg[ci][:], silu)
                        nc.vector.tensor_mul(h_sb[:, m, c0:c0 + 512], sil[:], pu[ci][:])
            else:
                fp8ci1 = (b <= 1)
                bfch = chunks[:1] if fp8ci1 else chunks
                if fp8ci1:
                    w8_sb = wpool.tile([128, 16, 2, 128], f8, tag="w8", bufs=2,
                                       name=f"w8_{b}_{m}")
                    nc.sync.dma_start(w8_sb[:], gup8[b, m])
                for half in range(2):
                    w_sb = _w_for(half)
                    for j in range(KD // 2):
                        k = half * (KD // 2) + j
                        lg = w_sb[:, (2 * j) * 128:(2 * j + 1) * 128]
                        lu = w_sb[:, (2 * j + 1) * 128:(2 * j + 2) * 128]
                        for ci, c0 in enumerate(bfch):
                            nc.tensor.matmul(pg[ci][:, :], lg,
                                             x_sb[:, k, c0:c0 + 512],
                                             start=(k == 0), stop=(k == KD - 1))
                        for ci, c0 in enumerate(bfch):
                            nc.tensor.matmul(pu[ci][:, :], lu,
                                             x_sb[:, k, c0:c0 + 512],
                                             start=(k == 0), stop=(k == KD - 1))
                if fp8ci1:
                    # ci=1 gate/up as fp8 DoubleRow: contraction 256/instr,
                    # weights pre-scaled x64 into e4m3 range
                    for kk in range(8):
                        nc.tensor.matmul(pg[1][:, :], w8_sb[:, 2 * kk, :, :],
                                         x8_sb[:, kk, :, :], perf_mode=DR,
                                         start=(kk == 0), stop=(kk == 7))
                        nc.tensor.matmul(pu[1][:, :], w8_sb[:, 2 * kk + 1, :, :],
                                         x8_sb[:, kk, :, :], perf_mode=DR,
                                         start=(kk == 0), stop=(kk == 7))
                for ci, c0 in enumerate(chunks):
                    sil = tpool.tile([128, 512], f32, tag="t", name=f"s{b}_{m}_{ci}")
                    if fp8ci1 and ci == 1:
                        # undo the x64 weight scale so h stays true-scale
                        nc.scalar.activation(sil[:], pg[ci][:], silu, scale=1.0 / 64)
                        ut = tpool.tile([128, 512], f32, tag="t2", name=f"u{b}_{m}")
                        nc.scalar.activation(ut[:], pu[ci][:], copyf, scale=1.0 / 64)
                        nc.vector.tensor_mul(h_sb[:, m, c0:c0 + 512], sil[:], ut[:])
                    else:
                        nc.scalar.activation(sil[:], pg[ci][:], silu)
                        nc.vector.tensor_mul(h_sb[:, m, c0:c0 + 512], sil[:], pu[ci][:])

        # phase 2: down matmul -> yT (chunk-outer so the ci=0 PSUM drain
        # overlaps the ci=1 accumulation, shortening the kernel tail)
        last = (b == len(TBLOCKS) - 1)
        for m2 in range(KD):
            wd_sb = wpool.tile([128, F], bf16, tag="wd", bufs=4, name=f"wd{b}_{m2}")
            nc.sync.dma_start(wd_sb[:, :], wdp[b, m2, :, :])
            if not last:
                # prefetch the next block's x / first weights in ~1MB pieces,
                # one per m2 period, so the wd stream never starves behind a
                # prefetch burst on the DMA rings
                if 2 <= m2 <= 5:
                    _issue_x_piece(b + 1, (m2 - 2) * 4, (m2 - 1) * 4)
                elif m2 == 6:
                    _issue_w(b + 1, 0)
                elif m2 == 7:
                    _issue_w(b + 1, 1)
            o_sb = opool.tile([128, T], bf16, tag="o", name=f"o{b}_{m2}")
            fin = last and m2 == KD - 1
            for ci, c0 in enumerate(chunks):
                if fin and ci == nch - 1:
                    # final chunk: two N=256 half-chains on separate PSUM
                    # banks so the drain copy/DMA overlaps the last matmuls
                    for hh, ptag in ((0, P2TAGS[1][1]), (1, "psH")):
                        pd = pspool.tile([128, 256], f32, tag=ptag,
                                         name=f"pdf_{hh}")
                        hc = c0 + hh * 256
                        for kf in range(MF):
                            nc.tensor.matmul(pd[:, :],
                                             wd_sb[:, kf * 128:(kf + 1) * 128],
                                             h_sb[:, kf, hc:hc + 256],
                                             start=(kf == 0), stop=(kf == MF - 1))
                        nc.vector.tensor_copy(o_sb[:, hc:hc + 256], pd[:])
                        nc.sync.dma_start(
                            yT[m2 * 128:(m2 + 1) * 128, off + hc:off + hc + 256],
                            o_sb[:, hc:hc + 256])
                    continue
                pd = pspool.tile([128, 512], f32, tag=P2TAGS[m2 % 2][ci],
                                 name=f"pd{b}_{m2}_{ci}")
                for kf in range(MF):
                    nc.tensor.matmul(pd[:, :], wd_sb[:, kf * 128:(kf + 1) * 128],
                                     h_sb[:, kf, c0:c0 + 512],
                                     start=(kf == 0), stop=(kf == MF - 1))
                nc.vector.tensor_copy(o_sb[:, c0:c0 + 512], pd[:])
                if last and m2 >= KD - 2:
                    # drain the final outputs per-chunk so the kernel tail
                    # doesn't wait on the second copy before starting the DMA
                    nc.sync.dma_start(
                        yT[m2 * 128:(m2 + 1) * 128, off + c0:off + c0 + 512],
                        o_sb[:, c0:c0 + 512])
            if not (last and m2 >= KD - 2):
                nc.sync.dma_start(
                    yT[m2 * 128:(m2 + 1) * 128, off:off + T], o_sb[:])


def _build_program():
    from contextlib import ExitStack

    import concourse.tile as tile
    from concourse import bacc, mybir

    nc = bacc.Bacc("TRN2", target_bir_lowering=False, debug=False,
                   enable_asserts=False, num_devices=NCORES)
    f32 = mybir.dt.float32
    xT = nc.dram_tensor("xT", [128, KD, TCOLS], mybir.dt.bfloat16, kind="ExternalInput").ap()
    gup = nc.dram_tensor("gup", [3, MF, 128, KD * 256], mybir.dt.bfloat16, kind="ExternalInput").ap()
    wdp = nc.dram_tensor("wdp", [3, KD, 128, F], mybir.dt.bfloat16, kind="ExternalInput").ap()
    xq8 = nc.dram_tensor("xq8", [2, 128, 8, 2, 512], mybir.dt.float8e4, kind="ExternalInput").ap()
    gup8 = nc.dram_tensor("gup8", [2, MF, 128, 16, 2, 128], mybir.dt.float8e4, kind="ExternalInput").ap()
    yT = nc.dram_tensor("yT", [D, TCOLS], mybir.dt.bfloat16, kind="ExternalOutput").ap()

    with tile.TileContext(nc) as tc, ExitStack() as ctx:
        _emit(ctx, tc, xT, gup, wdp, yT, xq8, gup8)
    nc.compile()
    return nc


def _get_program():
    if "nc" not in _prog_cache:
        _prog_cache["nc"] = _build_program()
    return _prog_cache["nc"]


def _pack_gu(wg, wu):
    # [F, D] x2 -> [MF, 128, KD*2*128]; tile [:, :, (k*2+g)*128 + f]
    g = wg.reshape(MF, 128, KD, 128).transpose(0, 3, 2, 1)   # [m, p, k, f]
    u = wu.reshape(MF, 128, KD, 128).transpose(0, 3, 2, 1)
    return np.ascontiguousarray(
        np.stack([g, u], axis=3).reshape(MF, 128, KD * 256))


def _pack_wd(wd):
    # [D, F] -> [KD, 128, F]; tile [:, :, kf*128 + j] = wd[m2*128+j, kf*128+p]
    return np.ascontiguousarray(
        wd.reshape(KD, 128, MF, 128).transpose(0, 3, 2, 1).reshape(KD, 128, F))


def _swiglu_np(x, wg, wu, wd):
    # numpy fallback for capacity overflow (float32, matches reference math)
    a = x @ wg.T
    h = (a / (1.0 + np.exp(-a))) * (x @ wu.T)
    return h @ wd.T


def _ensure_axon_hooks():
    """Make ``antenv.axon_hooks`` importable (bass_utils needs it when
    BASS_TRACE=1 under axon; some images ship antenv without it)."""
    try:
        import antenv.axon_hooks  # noqa: F401
        return
    except ImportError:
        pass
    import sys
    import types

    mod = types.ModuleType("antenv.axon_hooks")
    mod._hook = None

    def set_axon_ntff_profile_hook(h):
        mod._hook = h

    def get_axon_ntff_profile_hook():
        return mod._hook

    mod.set_axon_ntff_profile_hook = set_axon_ntff_profile_hook
    mod.get_axon_ntff_profile_hook = get_axon_ntff_profile_hook
    try:
        import antenv

        sys.modules["antenv.axon_hooks"] = mod
        antenv.axon_hooks = mod
    except ImportError:
        return
    try:
        from trn_agent_boot.trn_boot import _ntff_profile_via_ctypes

        mod._hook = _ntff_profile_via_ctypes("/opt/axon/libaxon_pjrt.so")
    except Exception:
        pass


def kernel(x, gate_w, w_gate, w_up, w_down, sw_gate, sw_up, sw_down, expert_bias):
    from concourse.bass_utils import run_bass_kernel_spmd

    _ensure_axon_hooks()

    x = np.asarray(x, np.float32)
    gate_w = np.asarray(gate_w, np.float32)
    w_gate = np.asarray(w_gate, np.float32)
    w_up = np.asarray(w_up, np.float32)
    w_down = np.asarray(w_down, np.float32)
    sw_gate = np.asarray(sw_gate, np.float32)
    sw_up = np.asarray(sw_up, np.float32)
    sw_down = np.asarray(sw_down, np.float32)
    expert_bias = np.asarray(expert_bias, np.float32)

    flat = x.reshape(N, D)
    bf16 = __import__("concourse.mybir", fromlist=["dt"]).dt.np(
        __import__("concourse.mybir", fromlist=["dt"]).dt.bfloat16)

    # ---- host gating / routing ----
    logits = flat.astype(np.float64) @ gate_w.astype(np.float64).T
    biased = logits + expert_bias.astype(np.float64)[None, :]
    order = np.argsort(-biased, axis=1, kind="stable")
    top_idx = order[:, :TOPK]                                  # [N, 2]
    m64 = logits.max(axis=1, keepdims=True)
    p = np.exp(logits - m64)
    probs = p / p.sum(axis=1, keepdims=True)
    top_w = np.take_along_axis(probs, top_idx, axis=1)
    top_w = top_w / top_w.sum(axis=1, keepdims=True)           # [N, 2]

    idx_list, w_list = [], []
    for e in range(E):
        sel = (top_idx == e)
        rows = np.where(sel.any(axis=1))[0]
        we = np.where(sel[rows, 0], top_w[rows, 0], top_w[rows, 1]).astype(np.float32)
        # weight-descending order: the fp8 ci=1 slots (512:1024) then hold
        # the lowest-combine-weight tokens, minimizing quantization energy
        osrt = np.argsort(-we, kind="stable")
        idx_list.append(rows[osrt])
        w_list.append(we[osrt])

    # ---- build per-core inputs ----
    sw_gu = _pack_gu(sw_gate, sw_up)
    sw_d = _pack_wd(sw_down)
    in_maps = []
    for c in range(NCORES):
        xT = np.zeros((D, TCOLS), bf16)  # packed to [128, KD, TCOLS] below
        gu = np.empty((3, MF, 128, KD * 256), bf16)
        wd = np.empty((3, KD, 128, F), bf16)
        for half in range(2):
            e = 2 * c + half
            rows = idx_list[e][:C]
            xT[:, half * C:half * C + len(rows)] = flat[rows].T
            gu[half] = _pack_gu(w_gate[e], w_up[e])
            wd[half] = _pack_wd(w_down[e])
        xT[:, 2 * C:] = flat[c * NS:(c + 1) * NS].T
        gu[2] = sw_gu
        wd[2] = sw_d
        xTp = np.ascontiguousarray(
            xT.reshape(KD, 128, TCOLS).transpose(1, 0, 2))
        f8np = __import__("concourse.mybir", fromlist=["dt"]).dt.np(
            __import__("concourse.mybir", fromlist=["dt"]).dt.float8e4)
        x8 = np.empty((2, 128, 8, 2, 512), f8np)
        gu8 = np.empty((2, MF, 128, 16, 2, 128), f8np)
        for hb in range(2):
            xf = np.asarray(xT[:, hb * C + 512:hb * C + 1024], np.float32)
            x8[hb] = np.ascontiguousarray(
                xf.reshape(8, 2, 128, 512).transpose(2, 0, 1, 3)).astype(f8np)
            e0 = 2 * c + hb
            g8 = (w_gate[e0] * 64.0).reshape(MF, 128, 8, 2, 128).transpose(0, 4, 2, 3, 1)
            u8 = (w_up[e0] * 64.0).reshape(MF, 128, 8, 2, 128).transpose(0, 4, 2, 3, 1)
            gu8[hb] = np.ascontiguousarray(
                np.stack([g8, u8], axis=3).reshape(MF, 128, 16, 2, 128)).astype(f8np)
        in_maps.append({"xT": xTp, "gup": gu, "wdp": wd, "xq8": x8, "gup8": gu8})

    # ---- run on 8 cores ----
    nc = _get_program()
    res = run_bass_kernel_spmd(nc, in_maps, core_ids=list(range(NCORES)))
    _prog_cache["last_results"] = res

    # ---- combine (expert-id order, then shared — matches reference) ----
    out = np.zeros((N, D), np.float32)
    for e in range(E):
        c, half = divmod(e, 2)
        rows, we = idx_list[e], w_list[e]
        ndev = min(len(rows), C)
        y = res.results[c]["yT"][:, half * C:half * C + ndev].T.astype(np.float32)
        out[rows[:ndev]] += we[:ndev, None] * y
        if len(rows) > C:  # capacity overflow: finish the tail on host
            r2 = rows[C:]
            y2 = _swiglu_np(flat[r2], w_gate[e], w_up[e], w_down[e])
            out[r2] += we[C:, None] * y2
    for c in range(NCORES):
        out[c * NS:(c + 1) * NS] += res.results[c]["yT"][:, 2 * C:].T.astype(np.float32)

    return out.reshape(B, S, D)

